# revision 28
# baseline (speedup 1.0000x reference)
"""Involution kernel for Trainium2, 8-core data-parallel (1 batch image per core).

Reference computation (per image, NHWC, C=64, G=4 groups, K=3, reduction 4):
    t    = relu(BN(x @ w1 + b1))            # [H,W,16]
    kern = t @ w2 + b2                      # [H,W,36], e = (ki*3+kj)*4 + g
    out[h,w,c] = sum_p kern[h,w, 4p + c%4] * xpad[h+di, w+dj, c]

v2 device strategy (single 128-subtile block, interior-only compute):
  * 128 subtiles of 24x12 interior, one per SBUF partition.  x2 holds the
    28x16 2-ring halo window per subtile; 3x3 shifts are free-dim offsets.
  * kern is generated ONLY at the 288 interior positions per subtile
    (q = 3j + q', j in 0..95, q' in 0..2).
  * mm1 streams xt [65, 36864] (columns ordered (cc, q', jl, st)); matmuls
    write a [96, 1536] psum at partition bases {0,32,64} (one 32-row block
    per q', rows 17..31 zeroed via the zero-padded w1x stationary, row 16 =
    ones via a selector column).  One relu evac per 1536-col chunk covers
    all three blocks -> tp2 [96, 12288] with cols (j, st).
  * mm2 is 96 block-diagonal matmuls: lhsT = tp2 128-col slice (stationary),
    rhs = BD3 [96, 108] = diag(w2b x3 at 32-row spacing), out = [128st, 108]
    = kern for 3 consecutive q.  4 j per psum bank, evac'd by ScalarE.
  * Involution on DVE in 4 chunks of 6 qi-rows (72 q): 9 bf16 tensor_tensor
    mults (kern broadcast over the 16 channels of each group via stride-0 AP)
    + 8 adds per chunk, overlapped with the next chunk's mm2/evac.
  * All bulk data bf16; host pre-builds layouts and folds BN into w1.
"""

import numpy as np
import ml_dtypes

import concourse.bass as bass
import concourse.bacc as bacc
import concourse.mybir as mybir
from concourse.tile import TileContext
from concourse.bass_utils import run_bass_kernel_spmd

BF16 = mybir.dt.bfloat16
NPF32 = np.float32
NPBF16 = ml_dtypes.bfloat16
AF = mybir.ActivationFunctionType

B, H, W, C = 8, 192, 192, 64
G, K, CR, E = 4, 3, 16, 36
BN_EPS = 1e-3
SH, SW = 24, 12            # subtile interior
NSI, NSJ = H // SH, W // SW  # 8 x 16 subtile grid -> 128 subtiles
NST = NSI * NSJ
NQ = SH * SW               # 288 interior positions per subtile
QG = 3                     # q-group: q = QG*j + q'
NJ = NQ // QG              # 96 block-diag matmuls
PP = 32 * QG               # 96 tp2 partitions (32-row block per q')
X2H, X2W = SH + 4, SW + 4  # 28, 16 (2-ring halo)
F2 = X2H * X2W * C         # 28672 x2 free elems per subtile
NPIX = NST * NQ            # 36864 pixel columns
NCC = 8                    # mm1 column chunks
CC_J = NJ // NCC           # 12 j per mm1 chunk
CC_COLS = CC_J * NST       # 1536 tp2 cols per mm1 chunk
CH_ROWS = (2, 11, 11)      # involution chunk qi-rows (small first: lead-in)
NCH = len(CH_ROWS)
CH_R0 = (0, 2, 13)         # chunk row offsets
# mm1 cc chunks that must complete before each involution chunk's mm2
CH_CCS = ((0, 1), (1, 5), (5, 8))
JB = 4                     # mm2 j's per psum bank (4*108 f32 = 1728B)

_CACHE = {}


def _build_program():
    if "nc" in _CACHE:
        return _CACHE["nc"]
    nc = bacc.Bacc(None, target_bir_lowering=False)
    x2_d = nc.dram_tensor("x2", [NST, F2], BF16, kind="ExternalInput")
    xt_d = nc.dram_tensor("xt", [C + 1, NPIX], BF16, kind="ExternalInput")
    # w1x and bd3 packed in one blob: cols 0:32 = w1x (rows 0:65), 32:140 = bd3
    wb_d = nc.dram_tensor("wb", [PP, 32 + QG * E], BF16, kind="ExternalInput")
    o_d = nc.dram_tensor("o", [NST, NQ * C], BF16, kind="ExternalOutput")

    with TileContext(nc) as tc:
        with (
            tc.tile_pool(name="const", bufs=1) as cpool,
            tc.tile_pool(name="x2p", bufs=1) as x2pool,
            tc.tile_pool(name="tpp", bufs=1) as tppool,
            tc.tile_pool(name="kernp", bufs=2) as kpool,
            tc.tile_pool(name="accp", bufs=2) as apool,
            tc.tile_pool(name="prodp", bufs=1) as ppool,
            tc.tile_pool(name="xtp", bufs=4) as xtpool,
            tc.tile_pool(name="ps1", bufs=2, space="PSUM") as ps1pool,
            tc.tile_pool(name="ps2", bufs=2, space="PSUM") as ps2pool,
        ):
            wbt = cpool.tile([PP, 32 + QG * E], BF16, tag="wb")
            w1t = wbt[0:C + 1, 0:32]
            bdt = wbt[:, 32:32 + QG * E]

            x2t = x2pool.tile([NST, F2], BF16, tag="x2")
            tp2 = tppool.tile([PP, NJ * NST], BF16, tag="tp2")
            x2v = x2t[:].rearrange("p (h w c) -> p h (w c)", h=X2H, c=C)

            xtts = {}

            def issue_xt(cc):
                xtt = xtpool.tile([C + 1, QG * CC_COLS], BF16, tag="xt")
                nc.sync.dma_start(
                    xtt[:],
                    xt_d[:, cc * QG * CC_COLS:(cc + 1) * QG * CC_COLS])
                xtts[cc] = xtt

            # DMA issue order: transfers complete roughly in issue order, so
            # latency-critical small loads go first; x2 pieces sized to each
            # involution chunk's halo rows interleave with the xt stream.
            r1, r2 = 5 * X2W * C, 13 * X2W * C
            issue_xt(0)
            nc.sync.dma_start(wbt[:], wb_d[:])
            nc.sync.dma_start(x2t[:, :r1], x2_d[:, :r1])
            issue_xt(1)
            issue_xt(2)
            issue_xt(3)
            nc.sync.dma_start(x2t[:, r1:r2], x2_d[:, r1:r2])
            nc.sync.dma_start(x2t[:, r2:], x2_d[:, r2:])

            def mm1_cc(cc):
                """mm1 + relu for 12 j's: tp2[32q'+k, j*128+st] = relu(x@w1x)"""
                if cc not in xtts:
                    issue_xt(cc)
                xtt = xtts.pop(cc)
                pst = ps1pool.tile([PP, CC_COLS], mybir.dt.float32, tag="ps1")
                for qp in range(QG):
                    for c1 in range(0, CC_COLS, 512):
                        nc.tensor.matmul(
                            pst[32 * qp:32 * qp + 32, c1:c1 + 512],
                            w1t,
                            xtt[:, qp * CC_COLS + c1:qp * CC_COLS + c1 + 512],
                            start=True, stop=True)
                nc.scalar.activation(
                    tp2[:, cc * CC_COLS:(cc + 1) * CC_COLS], pst[:], AF.Relu)

            def chunk(k):
                """mm2 (block-diag) -> kern, then DVE involution, then out."""
                nrow, r0 = CH_ROWS[k], CH_R0[k]
                nq = nrow * SW
                nj = nq // QG
                j0k = r0 * SW // QG
                kern = kpool.tile([NST, nq * E], BF16, tag="kern")
                for grp in range(nj // JB):
                    j0 = j0k + JB * grp
                    ps2 = ps2pool.tile([NST, JB * QG * E], mybir.dt.float32,
                                       tag="ps2")
                    for jl in range(JB):
                        nc.tensor.matmul(
                            ps2[:, jl * QG * E:(jl + 1) * QG * E],
                            tp2[:, (j0 + jl) * NST:(j0 + jl + 1) * NST],
                            bdt,
                            start=True, stop=True)
                    nc.scalar.copy(
                        kern[:, grp * JB * QG * E:(grp + 1) * JB * QG * E],
                        ps2[:])

                acc = apool.tile([NST, nq * C], BF16, tag="acc")
                kv = kern[:].rearrange("p (q e) -> p q e", e=E)
                q0 = r0 * SW
                for p in range(9):
                    di, dj = p // 3, p % 3
                    xop = x2v[:, 1 + di + r0:1 + di + r0 + nrow,
                              (1 + dj) * C:(1 + dj) * C + SW * C]
                    krep = kv[:, :, 4 * p:4 * p + 4].unsqueeze(2).broadcast_to(
                        [NST, nq, CR, 4])
                    if p == 0:
                        nc.vector.tensor_tensor(
                            acc[:], xop, krep, mybir.AluOpType.mult)
                        continue
                    prod = ppool.tile([NST, nq * C], BF16, tag="prod")
                    nc.vector.tensor_tensor(
                        prod[:], xop, krep, mybir.AluOpType.mult)
                    if k == NCH - 1 and p == 8:
                        # split the final add so most of the last out-DMA
                        # overlaps the remaining DVE work (shorter tail)
                        sp = (nrow - 3) * SW * C
                        nc.vector.tensor_tensor(
                            acc[:, :sp], acc[:, :sp], prod[:, :sp],
                            mybir.AluOpType.add)
                        nc.sync.dma_start(
                            o_d[:, q0 * C:q0 * C + sp], acc[:, :sp])
                        nc.vector.tensor_tensor(
                            acc[:, sp:], acc[:, sp:], prod[:, sp:],
                            mybir.AluOpType.add)
                        nc.sync.dma_start(
                            o_d[:, q0 * C + sp:(q0 + nq) * C], acc[:, sp:])
                    else:
                        nc.vector.tensor_tensor(
                            acc[:], acc[:], prod[:], mybir.AluOpType.add)

                if k != NCH - 1:
                    nc.sync.dma_start(
                        o_d[:, q0 * C:(q0 + nq) * C], acc[:])

            # interleave mm1 cc-chunks with mm2+involution chunks so the PE
            # reaches chunk k's mm2 as soon as its tp2 prerequisites exist
            for k in range(NCH):
                for cc in range(*CH_CCS[k]):
                    mm1_cc(cc)
                chunk(k)
    nc.compile()
    _CACHE["nc"] = nc
    return nc


def _host_prep(x, w1, b1, gamma, beta, mean, var, w2, b2):
    """Per-core input maps. x: [8,192,192,64] f32."""
    a = (gamma / np.sqrt(var + BN_EPS)).astype(NPF32)
    wb = np.zeros((PP, 32 + QG * E), dtype=NPF32)
    wb[:C, :CR] = w1 * a[None, :]
    wb[C, :CR] = b1 * a + (beta - mean * a)
    wb[C, CR] = 1.0  # ones-selector column -> tp2 ones rows
    for qp in range(QG):
        wb[32 * qp:32 * qp + CR, 32 + E * qp:32 + E * qp + E] = w2
        wb[32 * qp + CR, 32 + E * qp:32 + E * qp + E] = b2
    wb = wb.astype(NPBF16)

    xb = x.astype(NPBF16)
    in_maps = []
    for b in range(B):
        xi = xb[b]
        xp2 = np.zeros((H + 4, W + 4, C), dtype=NPBF16)
        xp2[2:-2, 2:-2] = xi
        s = xp2.strides
        win = np.lib.stride_tricks.as_strided(
            xp2, (NSI, NSJ, X2H, X2W, C),
            (s[0] * SH, s[1] * SW, s[0], s[1], s[2]))
        x2 = np.ascontiguousarray(win).reshape(NST, F2)
        # xt columns ordered (cc, q', jl, st): q = QG*(cc*CC_J + jl) + q'
        xv = xi.reshape(NSI, SH, NSJ, SW, C)
        xq = xv.transpose(4, 0, 2, 1, 3).reshape(C, NST, NQ)     # [c, st, q]
        xr = xq.reshape(C, NST, NJ, QG)                          # [c, st, j, q']
        xr = xr.reshape(C, NST, NCC, CC_J, QG)                   # [c, st, cc, jl, q']
        xr = xr.transpose(0, 2, 4, 3, 1)                         # [c, cc, q', jl, st]
        xt = np.empty((C + 1, NPIX), dtype=NPBF16)
        xt[:C] = np.ascontiguousarray(xr).reshape(C, NPIX)
        xt[C] = NPBF16(1.0)
        in_maps.append({"x2": x2, "xt": xt, "wb": wb})
    return in_maps


def kernel(x, w1, b1, gamma, beta, mean, var, w2, b2, _bench=None):
    nc = _build_program()
    in_maps = _host_prep(np.asarray(x), np.asarray(w1), np.asarray(b1),
                         np.asarray(gamma), np.asarray(beta), np.asarray(mean),
                         np.asarray(var), np.asarray(w2), np.asarray(b2))
    kw = dict(_bench) if _bench else {}
    res = run_bass_kernel_spmd(nc, in_maps, core_ids=list(range(B)), **kw)
    if _bench is not None:
        _bench["result"] = res
    out = np.empty((B, H, W, C), dtype=NPF32)
    for b in range(B):
        ob = res.results[b]["o"].reshape(NSI, NSJ, SH, SW, C).astype(NPF32)
        out[b] = ob.transpose(0, 2, 1, 3, 4).reshape(H, W, C)
    return out


# revision 30
# speedup vs baseline: 1.0739x; 1.0739x over previous
"""Involution kernel for Trainium2, 8-core data-parallel (1 batch image per core).

Reference computation (per image, NHWC, C=64, G=4 groups, K=3, reduction 4):
    t    = relu(BN(x @ w1 + b1))            # [H,W,16]
    kern = t @ w2 + b2                      # [H,W,36], e = (ki*3+kj)*4 + g
    out[h,w,c] = sum_p kern[h,w, 4p + c%4] * xpad[h+di, w+dj, c]

v2 device strategy (single 128-subtile block, interior-only compute):
  * 128 subtiles of 24x12 interior, one per SBUF partition.  x2 holds the
    28x16 2-ring halo window per subtile; 3x3 shifts are free-dim offsets.
  * kern is generated ONLY at the 288 interior positions per subtile
    (q = 3j + q', j in 0..95, q' in 0..2).
  * mm1 streams xt [65, 36864] (columns ordered (cc, q', jl, st)); matmuls
    write a [96, 1536] psum at partition bases {0,32,64} (one 32-row block
    per q', rows 17..31 zeroed via the zero-padded w1x stationary, row 16 =
    ones via a selector column).  One relu evac per 1536-col chunk covers
    all three blocks -> tp2 [96, 12288] with cols (j, st).
  * mm2 is 96 block-diagonal matmuls: lhsT = tp2 128-col slice (stationary),
    rhs = BD3 [96, 108] = diag(w2b x3 at 32-row spacing), out = [128st, 108]
    = kern for 3 consecutive q.  4 j per psum bank, evac'd by ScalarE.
  * Involution on DVE in 4 chunks of 6 qi-rows (72 q): 9 bf16 tensor_tensor
    mults (kern broadcast over the 16 channels of each group via stride-0 AP)
    + 8 adds per chunk, overlapped with the next chunk's mm2/evac.
  * All bulk data bf16; host pre-builds layouts and folds BN into w1.
"""

import numpy as np
import ml_dtypes

import concourse.bass as bass
import concourse.bacc as bacc
import concourse.mybir as mybir
from concourse.tile import TileContext
from concourse.bass_utils import run_bass_kernel_spmd

BF16 = mybir.dt.bfloat16
NPF32 = np.float32
NPBF16 = ml_dtypes.bfloat16
AF = mybir.ActivationFunctionType

B, H, W, C = 8, 192, 192, 64
G, K, CR, E = 4, 3, 16, 36
BN_EPS = 1e-3
SH, SW = 24, 12            # subtile interior
NSI, NSJ = H // SH, W // SW  # 8 x 16 subtile grid -> 128 subtiles
NST = NSI * NSJ
NQ = SH * SW               # 288 interior positions per subtile
QG = 3                     # q-group: q = QG*j + q'
NJ = NQ // QG              # 96 block-diag matmuls
PP = 32 * QG               # 96 tp2 partitions (32-row block per q')
X2H, X2W = SH + 4, SW + 4  # 28, 16 (2-ring halo)
F2 = X2H * X2W * C         # 28672 x2 free elems per subtile
NPIX = NST * NQ            # 36864 pixel columns
NCC = 8                    # mm1 column chunks
CC_J = NJ // NCC           # 12 j per mm1 chunk
CC_COLS = CC_J * NST       # 1536 tp2 cols per mm1 chunk
CH_ROWS = (2, 8, 8, 6)     # involution chunk qi-rows (small first: lead-in;
NCH = len(CH_ROWS)         #  small last: short final out-DMA tail)
CH_R0 = (0, 2, 10, 18)     # chunk row offsets
# mm1 cc chunks that must complete before each involution chunk's mm2
CH_CCS = ((0, 1), (1, 4), (4, 6), (6, 8))
JB = 4                     # mm2 j's per psum bank (4*108 f32 = 1728B)

_CACHE = {}


def _build_program():
    if "nc" in _CACHE:
        return _CACHE["nc"]
    nc = bacc.Bacc(None, target_bir_lowering=False)
    x2_d = nc.dram_tensor("x2", [NST, F2], BF16, kind="ExternalInput")
    xt_d = nc.dram_tensor("xt", [C + 1, NPIX], BF16, kind="ExternalInput")
    # w1x and bd3 packed in one blob: cols 0:32 = w1x (rows 0:65), 32:140 = bd3
    wb_d = nc.dram_tensor("wb", [PP, 32 + QG * E], BF16, kind="ExternalInput")
    o_d = nc.dram_tensor("o", [NST, NQ * C], BF16, kind="ExternalOutput")

    with TileContext(nc) as tc:
        with (
            tc.tile_pool(name="const", bufs=1) as cpool,
            tc.tile_pool(name="x2p", bufs=1) as x2pool,
            tc.tile_pool(name="tpp", bufs=1) as tppool,
            tc.tile_pool(name="kernp", bufs=2) as kpool,
            tc.tile_pool(name="accp", bufs=2) as apool,
            tc.tile_pool(name="prodp", bufs=1) as ppool,
            tc.tile_pool(name="xtp", bufs=4) as xtpool,
            tc.tile_pool(name="ps1", bufs=2, space="PSUM") as ps1pool,
            tc.tile_pool(name="ps2", bufs=2, space="PSUM") as ps2pool,
        ):
            wbt = cpool.tile([PP, 32 + QG * E], BF16, tag="wb")
            w1t = wbt[0:C + 1, 0:32]
            bdt = wbt[:, 32:32 + QG * E]

            x2t = x2pool.tile([NST, F2], BF16, tag="x2")
            tp2 = tppool.tile([PP, NJ * NST], BF16, tag="tp2")
            x2v = x2t[:].rearrange("p (h w c) -> p h (w c)", h=X2H, c=C)

            xtts = {}

            def issue_xt(cc):
                xtt = xtpool.tile([C + 1, QG * CC_COLS], BF16, tag="xt")
                nc.sync.dma_start(
                    xtt[:],
                    xt_d[:, cc * QG * CC_COLS:(cc + 1) * QG * CC_COLS])
                xtts[cc] = xtt

            # DMA issue order: transfers complete roughly in issue order, so
            # latency-critical small loads go first; x2 pieces sized to each
            # involution chunk's halo rows interleave with the xt stream.
            r1, r2 = 5 * X2W * C, 13 * X2W * C
            issue_xt(0)
            nc.sync.dma_start(wbt[:], wb_d[:])
            nc.sync.dma_start(x2t[:, :r1], x2_d[:, :r1])
            issue_xt(1)
            issue_xt(2)
            issue_xt(3)
            nc.sync.dma_start(x2t[:, r1:r2], x2_d[:, r1:r2])
            nc.sync.dma_start(x2t[:, r2:], x2_d[:, r2:])

            def mm1_cc(cc):
                """mm1 + relu for 12 j's: tp2[32q'+k, j*128+st] = relu(x@w1x)"""
                if cc not in xtts:
                    issue_xt(cc)
                xtt = xtts.pop(cc)
                pst = ps1pool.tile([PP, CC_COLS], mybir.dt.float32, tag="ps1")
                for qp in range(QG):
                    for c1 in range(0, CC_COLS, 512):
                        nc.tensor.matmul(
                            pst[32 * qp:32 * qp + 32, c1:c1 + 512],
                            w1t,
                            xtt[:, qp * CC_COLS + c1:qp * CC_COLS + c1 + 512],
                            start=True, stop=True)
                nc.scalar.activation(
                    tp2[:, cc * CC_COLS:(cc + 1) * CC_COLS], pst[:], AF.Relu)

            def chunk(k):
                """mm2 (block-diag) -> kern, then DVE involution, then out."""
                nrow, r0 = CH_ROWS[k], CH_R0[k]
                nq = nrow * SW
                nj = nq // QG
                j0k = r0 * SW // QG
                kern = kpool.tile([NST, nq * E], BF16, tag="kern")
                for grp in range(nj // JB):
                    j0 = j0k + JB * grp
                    ps2 = ps2pool.tile([NST, JB * QG * E], mybir.dt.float32,
                                       tag="ps2")
                    for jl in range(JB):
                        nc.tensor.matmul(
                            ps2[:, jl * QG * E:(jl + 1) * QG * E],
                            tp2[:, (j0 + jl) * NST:(j0 + jl + 1) * NST],
                            bdt,
                            start=True, stop=True)
                    nc.scalar.copy(
                        kern[:, grp * JB * QG * E:(grp + 1) * JB * QG * E],
                        ps2[:])

                acc = apool.tile([NST, nq * C], BF16, tag="acc")
                kv = kern[:].rearrange("p (q e) -> p q e", e=E)
                q0 = r0 * SW
                for p in range(9):
                    di, dj = p // 3, p % 3
                    xop = x2v[:, 1 + di + r0:1 + di + r0 + nrow,
                              (1 + dj) * C:(1 + dj) * C + SW * C]
                    krep = kv[:, :, 4 * p:4 * p + 4].unsqueeze(2).broadcast_to(
                        [NST, nq, CR, 4])
                    if p == 0:
                        nc.vector.tensor_tensor(
                            acc[:], xop, krep, mybir.AluOpType.mult)
                        continue
                    prod = ppool.tile([NST, nq * C], BF16, tag="prod")
                    nc.vector.tensor_tensor(
                        prod[:], xop, krep, mybir.AluOpType.mult)
                    if k == NCH - 1 and p == 8:
                        # split the final add so most of the last out-DMA
                        # overlaps the remaining DVE work (shorter tail)
                        sp = (nrow - 3) * SW * C
                        nc.vector.tensor_tensor(
                            acc[:, :sp], acc[:, :sp], prod[:, :sp],
                            mybir.AluOpType.add)
                        nc.sync.dma_start(
                            o_d[:, q0 * C:q0 * C + sp], acc[:, :sp])
                        nc.vector.tensor_tensor(
                            acc[:, sp:], acc[:, sp:], prod[:, sp:],
                            mybir.AluOpType.add)
                        nc.sync.dma_start(
                            o_d[:, q0 * C + sp:(q0 + nq) * C], acc[:, sp:])
                    else:
                        nc.vector.tensor_tensor(
                            acc[:], acc[:], prod[:], mybir.AluOpType.add)

                if k != NCH - 1:
                    nc.sync.dma_start(
                        o_d[:, q0 * C:(q0 + nq) * C], acc[:])

            # interleave mm1 cc-chunks with mm2+involution chunks so the PE
            # reaches chunk k's mm2 as soon as its tp2 prerequisites exist
            for k in range(NCH):
                for cc in range(*CH_CCS[k]):
                    mm1_cc(cc)
                chunk(k)
    nc.compile()
    _CACHE["nc"] = nc
    return nc


def _host_prep(x, w1, b1, gamma, beta, mean, var, w2, b2):
    """Per-core input maps. x: [8,192,192,64] f32."""
    a = (gamma / np.sqrt(var + BN_EPS)).astype(NPF32)
    wb = np.zeros((PP, 32 + QG * E), dtype=NPF32)
    wb[:C, :CR] = w1 * a[None, :]
    wb[C, :CR] = b1 * a + (beta - mean * a)
    wb[C, CR] = 1.0  # ones-selector column -> tp2 ones rows
    for qp in range(QG):
        wb[32 * qp:32 * qp + CR, 32 + E * qp:32 + E * qp + E] = w2
        wb[32 * qp + CR, 32 + E * qp:32 + E * qp + E] = b2
    wb = wb.astype(NPBF16)

    xb = x.astype(NPBF16)
    in_maps = []
    for b in range(B):
        xi = xb[b]
        xp2 = np.zeros((H + 4, W + 4, C), dtype=NPBF16)
        xp2[2:-2, 2:-2] = xi
        s = xp2.strides
        win = np.lib.stride_tricks.as_strided(
            xp2, (NSI, NSJ, X2H, X2W, C),
            (s[0] * SH, s[1] * SW, s[0], s[1], s[2]))
        x2 = np.ascontiguousarray(win).reshape(NST, F2)
        # xt columns ordered (cc, q', jl, st): q = QG*(cc*CC_J + jl) + q'
        xv = xi.reshape(NSI, SH, NSJ, SW, C)
        xq = xv.transpose(4, 0, 2, 1, 3).reshape(C, NST, NQ)     # [c, st, q]
        xr = xq.reshape(C, NST, NJ, QG)                          # [c, st, j, q']
        xr = xr.reshape(C, NST, NCC, CC_J, QG)                   # [c, st, cc, jl, q']
        xr = xr.transpose(0, 2, 4, 3, 1)                         # [c, cc, q', jl, st]
        xt = np.empty((C + 1, NPIX), dtype=NPBF16)
        xt[:C] = np.ascontiguousarray(xr).reshape(C, NPIX)
        xt[C] = NPBF16(1.0)
        in_maps.append({"x2": x2, "xt": xt, "wb": wb})
    return in_maps


def kernel(x, w1, b1, gamma, beta, mean, var, w2, b2, _bench=None):
    nc = _build_program()
    in_maps = _host_prep(np.asarray(x), np.asarray(w1), np.asarray(b1),
                         np.asarray(gamma), np.asarray(beta), np.asarray(mean),
                         np.asarray(var), np.asarray(w2), np.asarray(b2))
    kw = dict(_bench) if _bench else {}
    res = run_bass_kernel_spmd(nc, in_maps, core_ids=list(range(B)), **kw)
    if _bench is not None:
        _bench["result"] = res
    out = np.empty((B, H, W, C), dtype=NPF32)
    for b in range(B):
        ob = res.results[b]["o"].reshape(NSI, NSJ, SH, SW, C).astype(NPF32)
        out[b] = ob.transpose(0, 2, 1, 3, 4).reshape(H, W, C)
    return out


# revision 31
# speedup vs baseline: 1.1477x; 1.0687x over previous
"""Involution kernel for Trainium2, 8-core data-parallel (1 batch image per core).

Reference computation (per image, NHWC, C=64, G=4 groups, K=3, reduction 4):
    t    = relu(BN(x @ w1 + b1))            # [H,W,16]
    kern = t @ w2 + b2                      # [H,W,36], e = (ki*3+kj)*4 + g
    out[h,w,c] = sum_p kern[h,w, 4p + c%4] * xpad[h+di, w+dj, c]

v2 device strategy (single 128-subtile block, interior-only compute):
  * 128 subtiles of 24x12 interior, one per SBUF partition.  x2 holds the
    28x16 2-ring halo window per subtile; 3x3 shifts are free-dim offsets.
  * kern is generated ONLY at the 288 interior positions per subtile
    (q = 3j + q', j in 0..95, q' in 0..2).
  * mm1 streams xt [65, 36864] (columns ordered (cc, q', jl, st)); matmuls
    write a [96, 1536] psum at partition bases {0,32,64} (one 32-row block
    per q', rows 17..31 zeroed via the zero-padded w1x stationary, row 16 =
    ones via a selector column).  One relu evac per 1536-col chunk covers
    all three blocks -> tp2 [96, 12288] with cols (j, st).
  * mm2 is 96 block-diagonal matmuls: lhsT = tp2 128-col slice (stationary),
    rhs = BD3 [96, 108] = diag(w2b x3 at 32-row spacing), out = [128st, 108]
    = kern for 3 consecutive q.  4 j per psum bank, evac'd by ScalarE.
  * Involution on DVE in 4 chunks of 6 qi-rows (72 q): 9 bf16 tensor_tensor
    mults (kern broadcast over the 16 channels of each group via stride-0 AP)
    + 8 adds per chunk, overlapped with the next chunk's mm2/evac.
  * All bulk data bf16; host pre-builds layouts and folds BN into w1.
"""

import numpy as np
import ml_dtypes

import concourse.bass as bass
import concourse.bacc as bacc
import concourse.mybir as mybir
from concourse.tile import TileContext
from concourse.bass_utils import run_bass_kernel_spmd

BF16 = mybir.dt.bfloat16
NPF32 = np.float32
NPBF16 = ml_dtypes.bfloat16
AF = mybir.ActivationFunctionType

B, H, W, C = 8, 192, 192, 64
G, K, CR, E = 4, 3, 16, 36
BN_EPS = 1e-3
SH, SW = 24, 12            # subtile interior
NSI, NSJ = H // SH, W // SW  # 8 x 16 subtile grid -> 128 subtiles
NST = NSI * NSJ
NQ = SH * SW               # 288 interior positions per subtile
QG = 3                     # q-group: q = QG*j + q'
NJ = NQ // QG              # 96 block-diag matmuls
PP = 32 * QG               # 96 tp2 partitions (32-row block per q')
X2H, X2W = SH + 4, SW + 4  # 28, 16 (2-ring halo)
F2 = X2H * X2W * C         # 28672 x2 free elems per subtile
NPIX = NST * NQ            # 36864 pixel columns
NCC = 8                    # mm1 column chunks
CC_J = NJ // NCC           # 12 j per mm1 chunk
CC_COLS = CC_J * NST       # 1536 tp2 cols per mm1 chunk
CH_ROWS = (2, 8, 8, 6)     # involution chunk qi-rows (small first: lead-in;
NCH = len(CH_ROWS)         #  small last: short final out-DMA tail)
CH_R0 = (0, 2, 10, 18)     # chunk row offsets
# mm1 cc chunks that must complete before each involution chunk's mm2
CH_CCS = ((0, 1), (1, 4), (4, 6), (6, 8))
JB = 4                     # mm2 j's per psum bank (4*108 f32 = 1728B)

_CACHE = {}


def _build_program():
    if "nc" in _CACHE:
        return _CACHE["nc"]
    nc = bacc.Bacc(None, target_bir_lowering=False)
    x2_d = nc.dram_tensor("x2", [NST, F2], BF16, kind="ExternalInput")
    xt_d = nc.dram_tensor("xt", [C + 1, NPIX], BF16, kind="ExternalInput")
    # w1x and bd3 packed in one blob: cols 0:32 = w1x (rows 0:65), 32:140 = bd3
    wb_d = nc.dram_tensor("wb", [PP, 32 + QG * E], BF16, kind="ExternalInput")
    id_d = nc.dram_tensor("idm", [NST, NST], BF16, kind="ExternalInput")
    o_d = nc.dram_tensor("o", [NST, NQ * C], BF16, kind="ExternalOutput")

    with TileContext(nc) as tc:
        with (
            tc.tile_pool(name="const", bufs=1) as cpool,
            tc.tile_pool(name="x2p", bufs=1) as x2pool,
            tc.tile_pool(name="tpp", bufs=1) as tppool,
            tc.tile_pool(name="kernp", bufs=2) as kpool,
            tc.tile_pool(name="accp", bufs=2) as apool,
            tc.tile_pool(name="prodp", bufs=1) as ppool,
            tc.tile_pool(name="dpp", bufs=1) as dpool,
            tc.tile_pool(name="xtp", bufs=4) as xtpool,
            tc.tile_pool(name="ps1", bufs=1, space="PSUM") as ps1pool,
            tc.tile_pool(name="psa", bufs=3, space="PSUM") as psapool,
            tc.tile_pool(name="ps2", bufs=2, space="PSUM") as ps2pool,
        ):
            wbt = cpool.tile([PP, 32 + QG * E], BF16, tag="wb")
            w1t = wbt[0:C + 1, 0:32]
            bdt = wbt[:, 32:32 + QG * E]

            x2t = x2pool.tile([NST, F2], BF16, tag="x2")
            tp2 = tppool.tile([PP, NJ * NST], BF16, tag="tp2")
            x2v = x2t[:].rearrange("p (h w c) -> p h (w c)", h=X2H, c=C)

            xtts = {}

            def issue_xt(cc):
                xtt = xtpool.tile([C + 1, QG * CC_COLS], BF16, tag="xt")
                nc.sync.dma_start(
                    xtt[:],
                    xt_d[:, cc * QG * CC_COLS:(cc + 1) * QG * CC_COLS])
                xtts[cc] = xtt

            # DMA issue order: transfers complete roughly in issue order, so
            # latency-critical small loads go first; x2 pieces sized to each
            # involution chunk's halo rows interleave with the xt stream.
            r1, r2 = 5 * X2W * C, 13 * X2W * C
            issue_xt(0)
            nc.sync.dma_start(wbt[:], wb_d[:])
            nc.sync.dma_start(x2t[:, :r1], x2_d[:, :r1])
            issue_xt(1)
            issue_xt(2)
            issue_xt(3)
            nc.sync.dma_start(x2t[:, r1:r2], x2_d[:, r1:r2])
            nc.sync.dma_start(x2t[:, r2:], x2_d[:, r2:])
            idt = cpool.tile([NST, NST], BF16, tag="idm")
            nc.sync.dma_start(idt[:], id_d[:])

            def mm1_cc(cc):
                """mm1 + relu for 12 j's: tp2[32q'+k, j*128+st] = relu(x@w1x)"""
                if cc not in xtts:
                    issue_xt(cc)
                xtt = xtts.pop(cc)
                pst = ps1pool.tile([PP, CC_COLS], mybir.dt.float32, tag="ps1")
                for qp in range(QG):
                    for c1 in range(0, CC_COLS, 512):
                        nc.tensor.matmul(
                            pst[32 * qp:32 * qp + 32, c1:c1 + 512],
                            w1t,
                            xtt[:, qp * CC_COLS + c1:qp * CC_COLS + c1 + 512],
                            start=True, stop=True)
                nc.scalar.activation(
                    tp2[:, cc * CC_COLS:(cc + 1) * CC_COLS], pst[:], AF.Relu)

            kerns = {}

            def kern_gen(k):
                """mm2 (block-diag) -> kern for chunk k."""
                nrow, r0 = CH_ROWS[k], CH_R0[k]
                nq = nrow * SW
                nj = nq // QG
                j0k = r0 * SW // QG
                kern = kpool.tile([NST, nq * E], BF16, tag="kern")
                for grp in range(nj // JB):
                    j0 = j0k + JB * grp
                    ps2 = ps2pool.tile([NST, JB * QG * E], mybir.dt.float32,
                                       tag="ps2")
                    for jl in range(JB):
                        nc.tensor.matmul(
                            ps2[:, jl * QG * E:(jl + 1) * QG * E],
                            tp2[:, (j0 + jl) * NST:(j0 + jl + 1) * NST],
                            bdt,
                            start=True, stop=True)
                    nc.scalar.copy(
                        kern[:, grp * JB * QG * E:(grp + 1) * JB * QG * E],
                        ps2[:])
                kerns[k] = kern

            def invol(k):
                """DVE involution for chunk k (+PE tap-7/8 fold), then out."""
                nrow, r0 = CH_ROWS[k], CH_R0[k]
                nq = nrow * SW
                kern = kerns.pop(k)
                pe_fold = k in (1, 2)
                acc = apool.tile([NST, nq * C], BF16, tag="acc")
                kv = kern[:].rearrange("p (q e) -> p q e", e=E)
                q0 = r0 * SW
                dps = []
                for p in range(9):
                    di, dj = p // 3, p % 3
                    xop = x2v[:, 1 + di + r0:1 + di + r0 + nrow,
                              (1 + dj) * C:(1 + dj) * C + SW * C]
                    krep = kv[:, :, 4 * p:4 * p + 4].unsqueeze(2).broadcast_to(
                        [NST, nq, CR, 4])
                    if p == 0:
                        nc.vector.tensor_tensor(
                            acc[:], xop, krep, mybir.AluOpType.mult)
                        continue
                    if pe_fold and p >= 7:
                        dp = dpool.tile([NST, nq * C], BF16, tag=f"pd{p}")
                        nc.vector.tensor_tensor(
                            dp[:], xop, krep, mybir.AluOpType.mult)
                        dps.append(dp)
                        continue
                    prod = ppool.tile([NST, nq * C], BF16, tag="prod")
                    nc.vector.tensor_tensor(
                        prod[:], xop, krep, mybir.AluOpType.mult)
                    if k == NCH - 1 and p == 8:
                        # split the final add so most of the last out-DMA
                        # overlaps the remaining DVE work (shorter tail)
                        sp = (nrow - 3) * SW * C
                        nc.vector.tensor_tensor(
                            acc[:, :sp], acc[:, :sp], prod[:, :sp],
                            mybir.AluOpType.add)
                        nc.sync.dma_start(
                            o_d[:, q0 * C:q0 * C + sp], acc[:, :sp])
                        nc.vector.tensor_tensor(
                            acc[:, sp:], acc[:, sp:], prod[:, sp:],
                            mybir.AluOpType.add)
                        nc.sync.dma_start(
                            o_d[:, q0 * C + sp:(q0 + nq) * C], acc[:, sp:])
                    else:
                        nc.vector.tensor_tensor(
                            acc[:], acc[:], prod[:], mybir.AluOpType.add)

                if pe_fold:
                    # PE folds acc + prod7 + prod8 into fp32 PSUM pieces
                    # (identity-stationary matmuls); ScalarE evacs to acc.
                    for i in range(0, nq * C, 512):
                        psa = psapool.tile([NST, 512], mybir.dt.float32,
                                           tag="psa")
                        for si, s in enumerate([acc] + dps):
                            nc.tensor.matmul(
                                psa[:], idt[:], s[:, i:i + 512],
                                start=(si == 0), stop=(si == 2))
                        nc.scalar.copy(acc[:, i:i + 512], psa[:])
                if k != NCH - 1:
                    nc.sync.dma_start(
                        o_d[:, q0 * C:(q0 + nq) * C], acc[:])

            # interleave mm1 cc-chunks with mm2+involution chunks; kern_gen
            # runs one chunk ahead so the in-order PE queue reaches chunk
            # k+1's mm2 before chunk k's tap-fold matmuls
            for cc in range(*CH_CCS[0]):
                mm1_cc(cc)
            kern_gen(0)
            for k in range(NCH):
                if k + 1 < NCH:
                    for cc in range(*CH_CCS[k + 1]):
                        mm1_cc(cc)
                    kern_gen(k + 1)
                invol(k)
    nc.compile()
    _CACHE["nc"] = nc
    return nc


def _host_prep(x, w1, b1, gamma, beta, mean, var, w2, b2):
    """Per-core input maps. x: [8,192,192,64] f32."""
    a = (gamma / np.sqrt(var + BN_EPS)).astype(NPF32)
    wb = np.zeros((PP, 32 + QG * E), dtype=NPF32)
    wb[:C, :CR] = w1 * a[None, :]
    wb[C, :CR] = b1 * a + (beta - mean * a)
    wb[C, CR] = 1.0  # ones-selector column -> tp2 ones rows
    for qp in range(QG):
        wb[32 * qp:32 * qp + CR, 32 + E * qp:32 + E * qp + E] = w2
        wb[32 * qp + CR, 32 + E * qp:32 + E * qp + E] = b2
    wb = wb.astype(NPBF16)
    idm = np.eye(NST, dtype=NPF32).astype(NPBF16)

    xb = x.astype(NPBF16)
    in_maps = []
    for b in range(B):
        xi = xb[b]
        xp2 = np.zeros((H + 4, W + 4, C), dtype=NPBF16)
        xp2[2:-2, 2:-2] = xi
        s = xp2.strides
        win = np.lib.stride_tricks.as_strided(
            xp2, (NSI, NSJ, X2H, X2W, C),
            (s[0] * SH, s[1] * SW, s[0], s[1], s[2]))
        x2 = np.ascontiguousarray(win).reshape(NST, F2)
        # xt columns ordered (cc, q', jl, st): q = QG*(cc*CC_J + jl) + q'
        xv = xi.reshape(NSI, SH, NSJ, SW, C)
        xq = xv.transpose(4, 0, 2, 1, 3).reshape(C, NST, NQ)     # [c, st, q]
        xr = xq.reshape(C, NST, NJ, QG)                          # [c, st, j, q']
        xr = xr.reshape(C, NST, NCC, CC_J, QG)                   # [c, st, cc, jl, q']
        xr = xr.transpose(0, 2, 4, 3, 1)                         # [c, cc, q', jl, st]
        xt = np.empty((C + 1, NPIX), dtype=NPBF16)
        xt[:C] = np.ascontiguousarray(xr).reshape(C, NPIX)
        xt[C] = NPBF16(1.0)
        in_maps.append({"x2": x2, "xt": xt, "wb": wb, "idm": idm})
    return in_maps


def kernel(x, w1, b1, gamma, beta, mean, var, w2, b2, _bench=None):
    nc = _build_program()
    in_maps = _host_prep(np.asarray(x), np.asarray(w1), np.asarray(b1),
                         np.asarray(gamma), np.asarray(beta), np.asarray(mean),
                         np.asarray(var), np.asarray(w2), np.asarray(b2))
    kw = dict(_bench) if _bench else {}
    res = run_bass_kernel_spmd(nc, in_maps, core_ids=list(range(B)), **kw)
    if _bench is not None:
        _bench["result"] = res
    out = np.empty((B, H, W, C), dtype=NPF32)
    for b in range(B):
        ob = res.results[b]["o"].reshape(NSI, NSJ, SH, SW, C).astype(NPF32)
        out[b] = ob.transpose(0, 2, 1, 3, 4).reshape(H, W, C)
    return out


# revision 32
# speedup vs baseline: 1.1946x; 1.0408x over previous
"""Involution kernel for Trainium2, 8-core data-parallel (1 batch image per core).

Reference computation (per image, NHWC, C=64, G=4 groups, K=3, reduction 4):
    t    = relu(BN(x @ w1 + b1))            # [H,W,16]
    kern = t @ w2 + b2                      # [H,W,36], e = (ki*3+kj)*4 + g
    out[h,w,c] = sum_p kern[h,w, 4p + c%4] * xpad[h+di, w+dj, c]

v2 device strategy (single 128-subtile block, interior-only compute):
  * 128 subtiles of 24x12 interior, one per SBUF partition.  x2 holds the
    28x16 2-ring halo window per subtile; 3x3 shifts are free-dim offsets.
  * kern is generated ONLY at the 288 interior positions per subtile
    (q = 3j + q', j in 0..95, q' in 0..2).
  * mm1 streams xt [65, 36864] (columns ordered (cc, q', jl, st)); matmuls
    write a [96, 1536] psum at partition bases {0,32,64} (one 32-row block
    per q', rows 17..31 zeroed via the zero-padded w1x stationary, row 16 =
    ones via a selector column).  One relu evac per 1536-col chunk covers
    all three blocks -> tp2 [96, 12288] with cols (j, st).
  * mm2 is 96 block-diagonal matmuls: lhsT = tp2 128-col slice (stationary),
    rhs = BD3 [96, 108] = diag(w2b x3 at 32-row spacing), out = [128st, 108]
    = kern for 3 consecutive q.  4 j per psum bank, evac'd by ScalarE.
  * Involution on DVE in 4 chunks of 6 qi-rows (72 q): 9 bf16 tensor_tensor
    mults (kern broadcast over the 16 channels of each group via stride-0 AP)
    + 8 adds per chunk, overlapped with the next chunk's mm2/evac.
  * All bulk data bf16; host pre-builds layouts and folds BN into w1.
"""

import numpy as np
import ml_dtypes

import concourse.bass as bass
import concourse.bacc as bacc
import concourse.mybir as mybir
from concourse.tile import TileContext
from concourse.bass_utils import run_bass_kernel_spmd

BF16 = mybir.dt.bfloat16
NPF32 = np.float32
NPBF16 = ml_dtypes.bfloat16
AF = mybir.ActivationFunctionType

B, H, W, C = 8, 192, 192, 64
G, K, CR, E = 4, 3, 16, 36
BN_EPS = 1e-3
SH, SW = 24, 12            # subtile interior
NSI, NSJ = H // SH, W // SW  # 8 x 16 subtile grid -> 128 subtiles
NST = NSI * NSJ
NQ = SH * SW               # 288 interior positions per subtile
QG = 3                     # q-group: q = QG*j + q'
NJ = NQ // QG              # 96 block-diag matmuls
PP = 32 * QG               # 96 tp2 partitions (32-row block per q')
X2H, X2W = SH + 4, SW + 4  # 28, 16 (2-ring halo)
F2 = X2H * X2W * C         # 28672 x2 free elems per subtile
NPIX = NST * NQ            # 36864 pixel columns
NCC = 8                    # mm1 column chunks
CC_J = NJ // NCC           # 12 j per mm1 chunk
CC_COLS = CC_J * NST       # 1536 tp2 cols per mm1 chunk
CH_ROWS = (2, 8, 8, 6)     # involution chunk qi-rows (small first: lead-in;
NCH = len(CH_ROWS)         #  small last: short final out-DMA tail)
CH_R0 = (0, 2, 10, 18)     # chunk row offsets
# mm1 cc chunks that must complete before each involution chunk's mm2
CH_CCS = ((0, 1), (1, 4), (4, 6), (6, 8))
JB = 4                     # mm2 j's per psum bank (4*108 f32 = 1728B)

_CACHE = {}


def _build_program():
    if "nc" in _CACHE:
        return _CACHE["nc"]
    nc = bacc.Bacc(None, target_bir_lowering=False)
    x2_d = nc.dram_tensor("x2", [NST, F2], BF16, kind="ExternalInput")
    xt_d = nc.dram_tensor("xt", [C + 1, NPIX], BF16, kind="ExternalInput")
    # w1x and bd3 packed in one blob: cols 0:32 = w1x (rows 0:65), 32:140 = bd3
    wb_d = nc.dram_tensor("wb", [PP, 32 + QG * E], BF16, kind="ExternalInput")
    id_d = nc.dram_tensor("idm", [NST, NST], BF16, kind="ExternalInput")
    o_d = nc.dram_tensor("o", [NST, NQ * C], BF16, kind="ExternalOutput")

    with TileContext(nc) as tc:
        with (
            tc.tile_pool(name="const", bufs=1) as cpool,
            tc.tile_pool(name="x2p", bufs=1) as x2pool,
            tc.tile_pool(name="tpp", bufs=1) as tppool,
            tc.tile_pool(name="kernp", bufs=2) as kpool,
            tc.tile_pool(name="accp", bufs=2) as apool,
            tc.tile_pool(name="prodp", bufs=1) as ppool,
            tc.tile_pool(name="dpp", bufs=1) as dpool,
            tc.tile_pool(name="xtp", bufs=3) as xtpool,
            tc.tile_pool(name="ps1", bufs=1, space="PSUM") as ps1pool,
            tc.tile_pool(name="psa", bufs=3, space="PSUM") as psapool,
            tc.tile_pool(name="ps2", bufs=2, space="PSUM") as ps2pool,
        ):
            wbt = cpool.tile([PP, 32 + QG * E], BF16, tag="wb")
            w1t = wbt[0:C + 1, 0:32]
            bdt = wbt[:, 32:32 + QG * E]

            x2t = x2pool.tile([NST, F2], BF16, tag="x2")
            tp2 = tppool.tile([PP, NJ * NST], BF16, tag="tp2")
            x2v = x2t[:].rearrange("p (h w c) -> p h (w c)", h=X2H, c=C)

            xtts = {}

            def issue_xt(cc):
                xtt = xtpool.tile([C + 1, QG * CC_COLS], BF16, tag="xt")
                nc.sync.dma_start(
                    xtt[:],
                    xt_d[:, cc * QG * CC_COLS:(cc + 1) * QG * CC_COLS])
                xtts[cc] = xtt

            # DMA issue order: transfers complete roughly in issue order, so
            # latency-critical small loads go first; x2 pieces sized to each
            # involution chunk's halo rows interleave with the xt stream.
            r1, r2 = 5 * X2W * C, 13 * X2W * C
            issue_xt(0)
            nc.sync.dma_start(wbt[:], wb_d[:])
            nc.sync.dma_start(x2t[:, :r1], x2_d[:, :r1])
            issue_xt(1)
            issue_xt(2)
            issue_xt(3)
            nc.sync.dma_start(x2t[:, r1:r2], x2_d[:, r1:r2])
            nc.sync.dma_start(x2t[:, r2:], x2_d[:, r2:])
            idt = cpool.tile([NST, NST], BF16, tag="idm")
            nc.sync.dma_start(idt[:], id_d[:])

            def mm1_cc(cc):
                """mm1 + relu for 12 j's: tp2[32q'+k, j*128+st] = relu(x@w1x)"""
                if cc not in xtts:
                    issue_xt(cc)
                xtt = xtts.pop(cc)
                pst = ps1pool.tile([PP, CC_COLS], mybir.dt.float32, tag="ps1")
                for qp in range(QG):
                    for c1 in range(0, CC_COLS, 512):
                        nc.tensor.matmul(
                            pst[32 * qp:32 * qp + 32, c1:c1 + 512],
                            w1t,
                            xtt[:, qp * CC_COLS + c1:qp * CC_COLS + c1 + 512],
                            start=True, stop=True)
                nc.scalar.activation(
                    tp2[:, cc * CC_COLS:(cc + 1) * CC_COLS], pst[:], AF.Relu)

            kerns = {}

            def kern_gen(k):
                """mm2 (block-diag) -> kern for chunk k."""
                nrow, r0 = CH_ROWS[k], CH_R0[k]
                nq = nrow * SW
                nj = nq // QG
                j0k = r0 * SW // QG
                kern = kpool.tile([NST, nq * E], BF16, tag="kern")
                for grp in range(nj // JB):
                    j0 = j0k + JB * grp
                    ps2 = ps2pool.tile([NST, JB * QG * E], mybir.dt.float32,
                                       tag="ps2")
                    for jl in range(JB):
                        nc.tensor.matmul(
                            ps2[:, jl * QG * E:(jl + 1) * QG * E],
                            tp2[:, (j0 + jl) * NST:(j0 + jl + 1) * NST],
                            bdt,
                            start=True, stop=True)
                    nc.scalar.copy(
                        kern[:, grp * JB * QG * E:(grp + 1) * JB * QG * E],
                        ps2[:])
                kerns[k] = kern

            def invol(k):
                """DVE involution for chunk k (+PE tap-7/8 fold), then out."""
                nrow, r0 = CH_ROWS[k], CH_R0[k]
                nq = nrow * SW
                kern = kerns.pop(k)
                pe_fold = k in (1, 2)
                acc = apool.tile([NST, nq * C], BF16, tag="acc")
                kv = kern[:].rearrange("p (q e) -> p q e", e=E)
                q0 = r0 * SW
                dps = []
                for p in range(9):
                    di, dj = p // 3, p % 3
                    xop = x2v[:, 1 + di + r0:1 + di + r0 + nrow,
                              (1 + dj) * C:(1 + dj) * C + SW * C]
                    krep = kv[:, :, 4 * p:4 * p + 4].unsqueeze(2).broadcast_to(
                        [NST, nq, CR, 4])
                    if p == 0:
                        nc.vector.tensor_tensor(
                            acc[:], xop, krep, mybir.AluOpType.mult)
                        continue
                    if pe_fold and p >= 6:
                        dp = dpool.tile([NST, nq * C], BF16, tag=f"pd{p}")
                        nc.vector.tensor_tensor(
                            dp[:], xop, krep, mybir.AluOpType.mult)
                        dps.append(dp)
                        continue
                    prod = ppool.tile([NST, nq * C], BF16, tag="prod")
                    nc.vector.tensor_tensor(
                        prod[:], xop, krep, mybir.AluOpType.mult)
                    if k == NCH - 1 and p == 8:
                        # split the final add so most of the last out-DMA
                        # overlaps the remaining DVE work (shorter tail)
                        sp = (nrow - 3) * SW * C
                        nc.vector.tensor_tensor(
                            acc[:, :sp], acc[:, :sp], prod[:, :sp],
                            mybir.AluOpType.add)
                        nc.sync.dma_start(
                            o_d[:, q0 * C:q0 * C + sp], acc[:, :sp])
                        nc.vector.tensor_tensor(
                            acc[:, sp:], acc[:, sp:], prod[:, sp:],
                            mybir.AluOpType.add)
                        nc.sync.dma_start(
                            o_d[:, q0 * C + sp:(q0 + nq) * C], acc[:, sp:])
                    else:
                        nc.vector.tensor_tensor(
                            acc[:], acc[:], prod[:], mybir.AluOpType.add)

                if pe_fold:
                    # PE folds acc + prod7 + prod8 into fp32 PSUM pieces
                    # (identity-stationary matmuls); ScalarE evacs to acc.
                    for i in range(0, nq * C, 512):
                        psa = psapool.tile([NST, 512], mybir.dt.float32,
                                           tag="psa")
                        for si, s in enumerate([acc] + dps):
                            nc.tensor.matmul(
                                psa[:], idt[:], s[:, i:i + 512],
                                start=(si == 0), stop=(si == 3))
                        nc.scalar.copy(acc[:, i:i + 512], psa[:])
                if k != NCH - 1:
                    nc.sync.dma_start(
                        o_d[:, q0 * C:(q0 + nq) * C], acc[:])

            # interleave mm1 cc-chunks with mm2+involution chunks; kern_gen
            # runs one chunk ahead so the in-order PE queue reaches chunk
            # k+1's mm2 before chunk k's tap-fold matmuls
            for cc in range(*CH_CCS[0]):
                mm1_cc(cc)
            kern_gen(0)
            for k in range(NCH):
                if k + 1 < NCH:
                    for cc in range(*CH_CCS[k + 1]):
                        mm1_cc(cc)
                    kern_gen(k + 1)
                invol(k)
    nc.compile()
    _CACHE["nc"] = nc
    return nc


def _host_prep(x, w1, b1, gamma, beta, mean, var, w2, b2):
    """Per-core input maps. x: [8,192,192,64] f32."""
    a = (gamma / np.sqrt(var + BN_EPS)).astype(NPF32)
    wb = np.zeros((PP, 32 + QG * E), dtype=NPF32)
    wb[:C, :CR] = w1 * a[None, :]
    wb[C, :CR] = b1 * a + (beta - mean * a)
    wb[C, CR] = 1.0  # ones-selector column -> tp2 ones rows
    for qp in range(QG):
        wb[32 * qp:32 * qp + CR, 32 + E * qp:32 + E * qp + E] = w2
        wb[32 * qp + CR, 32 + E * qp:32 + E * qp + E] = b2
    wb = wb.astype(NPBF16)
    idm = np.eye(NST, dtype=NPF32).astype(NPBF16)

    xb = x.astype(NPBF16)
    in_maps = []
    for b in range(B):
        xi = xb[b]
        xp2 = np.zeros((H + 4, W + 4, C), dtype=NPBF16)
        xp2[2:-2, 2:-2] = xi
        s = xp2.strides
        win = np.lib.stride_tricks.as_strided(
            xp2, (NSI, NSJ, X2H, X2W, C),
            (s[0] * SH, s[1] * SW, s[0], s[1], s[2]))
        x2 = np.ascontiguousarray(win).reshape(NST, F2)
        # xt columns ordered (cc, q', jl, st): q = QG*(cc*CC_J + jl) + q'
        xv = xi.reshape(NSI, SH, NSJ, SW, C)
        xq = xv.transpose(4, 0, 2, 1, 3).reshape(C, NST, NQ)     # [c, st, q]
        xr = xq.reshape(C, NST, NJ, QG)                          # [c, st, j, q']
        xr = xr.reshape(C, NST, NCC, CC_J, QG)                   # [c, st, cc, jl, q']
        xr = xr.transpose(0, 2, 4, 3, 1)                         # [c, cc, q', jl, st]
        xt = np.empty((C + 1, NPIX), dtype=NPBF16)
        xt[:C] = np.ascontiguousarray(xr).reshape(C, NPIX)
        xt[C] = NPBF16(1.0)
        in_maps.append({"x2": x2, "xt": xt, "wb": wb, "idm": idm})
    return in_maps


def kernel(x, w1, b1, gamma, beta, mean, var, w2, b2, _bench=None):
    nc = _build_program()
    in_maps = _host_prep(np.asarray(x), np.asarray(w1), np.asarray(b1),
                         np.asarray(gamma), np.asarray(beta), np.asarray(mean),
                         np.asarray(var), np.asarray(w2), np.asarray(b2))
    kw = dict(_bench) if _bench else {}
    res = run_bass_kernel_spmd(nc, in_maps, core_ids=list(range(B)), **kw)
    if _bench is not None:
        _bench["result"] = res
    out = np.empty((B, H, W, C), dtype=NPF32)
    for b in range(B):
        ob = res.results[b]["o"].reshape(NSI, NSJ, SH, SW, C).astype(NPF32)
        out[b] = ob.transpose(0, 2, 1, 3, 4).reshape(H, W, C)
    return out
